# revision 55
# baseline (speedup 1.0000x reference)
"""Trainium2 Bass kernel for pre-LN multi-head self-attention.

Module: y = LN(x); qkv = y @ w_qkv; attention(8 heads, dh=64); out = ao @ w_out
Shapes: x [4, 2048, 512], w_qkv [512, 1536], w_out [512, 512], fp32.

Sharding (8 cores): core c -> batch b = c//2, head-group g = c%2 (4 heads).
Each core computes LN + QKV (its head slice) + attention + a partial output
projection (its heads' rows of w_out); the host sums the two partials per batch.

Per-core dataflow (transpose-free except one PE transpose of y):
  LN in natural [tok, d] layout (bn_stats) -> PE-transpose y -> yT [d, tok]
  Q^T, K^T = w^T @ yT   (features on partitions -- natural lhsT layout)
  V natural [tok, feat] with a fused ones-column so attn@V also accumulates
  the softmax denominator (row 64 of the PSUM accumulator).
  scoresT [k, q] = K^T.T @ Q^T per 128-k-token block; exp on ACT (no max
  subtraction needed: fp32, well-scaled inputs); attn@V accumulates over
  k-blocks in PSUM; per-head 1/sumexp broadcast via a DRAM roundtrip DMA;
  output projection consumes aoT directly as lhsT.
ln_scale/ln_bias are folded into w_qkv on the host (w_eff = scale*W,
bias_row = bias@W added per-feature on device), so the device LN is pure
normalize.  Matmul operands are bf16 (PSUM accumulation stays fp32); the
second matmul of each same-weight pair sets ldweights=False to skip the
redundant PE weight reload.  Stage D runs a depth-2 software pipeline
(scores+exp for item i+2 issue before attn@V of item i) so the in-order
PE never stalls on the ACT exp; the output projection is interleaved
per q-half, and the final unit normalizes in 128-token chunks so the
last projection tiles pipeline with it.
"""

import sys

if "/opt/trn_rl_repo" not in sys.path:
    sys.path.insert(0, "/opt/trn_rl_repo")

from contextlib import ExitStack

import numpy as np

import concourse.bass as bass
import concourse.tile as tile
from concourse.masks import make_identity
from concourse import bacc, mybir
from concourse.bass_utils import run_bass_kernel_spmd

B, N, D = 4, 2048, 512
H, DH = 8, 64
HPC = 4                 # heads per core
FPC = HPC * DH          # 256 features per core
P = 128
NT = N // P             # 16 token tiles
DT = D // P             # 4 d tiles
NQ = N // 512           # 4 q-blocks of 512
EPS = 1e-6
SCALE = DH ** -0.5
F32 = mybir.dt.float32
F32R = mybir.dt.float32r
BF16 = mybir.dt.bfloat16
ALU = mybir.AluOpType
AFT = mybir.ActivationFunctionType




def build_kernel():
    nc = bacc.Bacc("TRN2", target_bir_lowering=False, debug=False)
    xb = nc.dram_tensor("xb", [N, D], F32, kind="ExternalInput").ap()
    wq = nc.dram_tensor("wq", [D, FPC], BF16, kind="ExternalInput").ap()
    wk = nc.dram_tensor("wk", [D, FPC], BF16, kind="ExternalInput").ap()
    wv = nc.dram_tensor("wv", [D, FPC], BF16, kind="ExternalInput").ap()
    wo = nc.dram_tensor("wo", [FPC, D], BF16, kind="ExternalInput").ap()
    bq = nc.dram_tensor("bq", [FPC], F32, kind="ExternalInput").ap()
    bk = nc.dram_tensor("bk", [FPC], F32, kind="ExternalInput").ap()
    bv = nc.dram_tensor("bv", [FPC], F32, kind="ExternalInput").ap()
    out = nc.dram_tensor("out", [N, D], F32, kind="ExternalOutput").ap()

    with tile.TileContext(nc, pool_alloc_mode="queue") as tc, ExitStack() as ctx:
        consts = ctx.enter_context(tc.tile_pool(name="consts", bufs=1))
        big = ctx.enter_context(tc.tile_pool(name="big", bufs=1))
        dram = ctx.enter_context(tc.tile_pool(name="dram", bufs=2, space="DRAM"))

        identity = consts.tile([P, P], BF16)
        make_identity(nc, identity)
        eps_t = consts.tile([P, 1], F32)
        nc.vector.memset(eps_t, EPS)

        yT = [big.tile([P, N], BF16, tag=f"yT{j}", name=f"yT{j}") for j in range(DT)]
        qT = [big.tile([P, N], BF16, tag=f"qT{j}", name=f"qT{j}") for j in range(2)]
        kT = [big.tile([P, N], BF16, tag=f"kT{j}", name=f"kT{j}") for j in range(2)]
        aoT = [big.tile([P, N], BF16, tag=f"aoT{j}", name=f"aoT{j}") for j in range(2)]
        v_sb = big.tile([P, NT, HPC, DH + 1], BF16)
        ones_col = consts.tile([P, 1], F32)
        nc.vector.memset(ones_col, 1.0)
        nc.vector.tensor_copy(
            v_sb[:, :, :, DH : DH + 1],
            ones_col[:, 0:1].to_broadcast((P, NT, HPC, 1)),
        )

        # ---- Stage A+B: LayerNorm + transpose y -> yT ----
        with tc.tile_pool(name="ln", bufs=3) as ln, tc.tile_pool(
            name="tp_psum", bufs=8, space="PSUM"
        ) as tpp:
            xs = []
            for i in range(NT):
                x_t = ln.tile([P, D], F32, tag="x", bufs=NT, name=f"x{i}")
                nc.sync.dma_start(out=x_t, in_=xb[i * P : (i + 1) * P, :])
                xs.append(x_t)
            for ig in range(NT // 4):  # groups of 4 token tiles
                y_ts = []
                for ii in range(4):
                    i = ig * 4 + ii
                    x_t = xs[i]
                    stats = ln.tile([P, 6], F32, tag="stats")
                    nc.vector.bn_stats(out=stats, in_=x_t)
                    mv = ln.tile([P, 2], F32, tag="mv")
                    nc.vector.bn_aggr(out=mv, in_=stats)
                    std = ln.tile([P, 1], F32, tag="std")
                    nc.scalar.activation(
                        out=std, in_=mv[:, 1:2], func=AFT.Sqrt, bias=eps_t[:, 0:1]
                    )
                    rstd = ln.tile([P, 1], F32, tag="rstd")
                    nc.vector.reciprocal(out=rstd, in_=std)
                    y_t = ln.tile([P, D], BF16, tag="y", bufs=6)
                    nc.vector.tensor_scalar(
                        out=y_t,
                        in0=x_t,
                        scalar1=mv[:, 0:1],
                        scalar2=rstd[:, 0:1],
                        op0=ALU.subtract,
                        op1=ALU.mult,
                    )
                    y_ts.append(y_t)
                for j in range(DT):
                    pt = tpp.tile([P, 512], BF16, tag="tp")
                    for ii in range(4):
                        nc.tensor.transpose(
                            pt[:, ii * P : (ii + 1) * P],
                            y_ts[ii][:, j * P : (j + 1) * P],
                            identity,
                        )
                    nc.scalar.activation(
                        out=yT[j][:, ig * 512 : (ig + 1) * 512],
                        in_=pt,
                        func=AFT.Copy,
                    )

        # weights: [d, f] -> sbuf [p, dt, f]
        w_q_sb = consts.tile([P, DT, FPC], BF16)
        nc.sync.dma_start(out=w_q_sb, in_=wq.rearrange("(t p) f -> p t f", p=P))
        w_k_sb = consts.tile([P, DT, FPC], BF16)
        nc.sync.dma_start(out=w_k_sb, in_=wk.rearrange("(t p) f -> p t f", p=P))
        w_v_sb = consts.tile([P, DT, FPC], BF16)
        nc.sync.dma_start(out=w_v_sb, in_=wv.rearrange("(t p) f -> p t f", p=P))
        w_o_sb = consts.tile([P, 2, D], BF16)
        nc.sync.dma_start(out=w_o_sb, in_=wo.rearrange("(t p) f -> p t f", p=P))
        bq_sb = consts.tile([P, 2], F32)
        nc.sync.dma_start(out=bq_sb, in_=bq.rearrange("(t p) -> p t", p=P))
        bk_sb = consts.tile([P, 2], F32)
        nc.sync.dma_start(out=bk_sb, in_=bk.rearrange("(t p) -> p t", p=P))
        bv_b = consts.tile([P, FPC], F32)
        bv_bcast = bass.AP(tensor=bv.tensor, offset=bv.offset, ap=[[0, P]] + list(bv.ap))
        nc.sync.dma_start(out=bv_b, in_=bv_bcast)

        # ---- Stage C: QKV projections ----
        with tc.tile_pool(name="c_psum", bufs=1, space="PSUM") as cpp, tc.tile_pool(
            name="v_psum", bufs=2, space="PSUM"
        ) as vpp:
            # j=0 groups first so heads 0/1 attention can start early; each
            # weight tile is loaded once and reused across the 4 token groups
            def qk_group(w_sb, b_sb, dstT, j):
                pss = [
                    cpp.tile([P, 512], F32, tag=f"qk{nt}", name=f"qk{nt}_{j}")
                    for nt in range(NQ)
                ]
                for dt in range(DT):
                    for nt in range(NQ):
                        mm = nc.tensor.matmul(
                            pss[nt],
                            lhsT=(w_sb[:, dt, j * P : (j + 1) * P]),
                            rhs=(yT[dt][:, nt * 512 : (nt + 1) * 512]),
                            start=(dt == 0),
                            stop=(dt == DT - 1),
                        )
                        if nt > 0:
                            mm.ins.ldweights = False
                for nt in range(NQ):
                    nc.scalar.activation(
                        out=dstT[j][:, nt * 512 : (nt + 1) * 512],
                        in_=pss[nt],
                        func=AFT.Identity,
                        bias=b_sb[:, j : j + 1],
                    )

            def v_group(irange):
                for i in irange:
                    ps = vpp.tile([P, FPC], F32, tag="v", name=f"v{i}")
                    for dt in range(DT):
                        nc.tensor.matmul(
                            ps,
                            lhsT=(yT[dt][:, i * P : (i + 1) * P]),
                            rhs=(w_v_sb[:, dt, :]),
                            start=(dt == 0),
                            stop=(dt == DT - 1),
                        )
                    nc.vector.tensor_tensor(
                        out=v_sb[:, i, :, 0:DH],
                        in0=ps.rearrange("p (h d) -> p h d", h=HPC),
                        in1=bv_b.rearrange("p (h d) -> p h d", h=HPC),
                        op=ALU.add,
                    )

            qk_group(w_k_sb, bk_sb, kT, 0)
            qk_group(w_q_sb, bq_sb, qT, 0)
            v_group(range(0, 8))
            qk_group(w_k_sb, bk_sb, kT, 1)
            qk_group(w_q_sb, bq_sb, qT, 1)
            v_group(range(8, NT))

        # ---- Stage D: attention, units of (q-half, head) pipelined ----
        QH = 1024
        with tc.tile_pool(name="sc_psum", bufs=2, space="PSUM") as scp, tc.tile_pool(
            name="ao_psum", bufs=1, space="PSUM"
        ) as aop, tc.tile_pool(name="exp_sb", bufs=6) as exps, tc.tile_pool(
            name="nrm", bufs=3
        ) as nrm, tc.tile_pool(
            name="o_psum", bufs=2, space="PSUM"
        ) as opp, tc.tile_pool(name="o_sb", bufs=3) as osb:
            items = [
                (qh, h, kb) for qh in range(2) for h in range(HPC) for kb in range(NT)
            ]
            ex_tiles = {}
            ao_tiles = {}

            def sc_exp(i):
                qh, h, kb = items[i]
                j, po = h // 2, (h % 2) * DH
                q0 = qh * QH
                sc = scp.tile([P, QH], F32, tag="sc", name=f"sc{i}")
                for c in range(2):
                    mm = nc.tensor.matmul(
                        sc[:, c * 512 : (c + 1) * 512],
                        lhsT=(kT[j][po : po + DH, kb * P : (kb + 1) * P]),
                        rhs=(qT[j][po : po + DH, q0 + c * 512 : q0 + (c + 1) * 512]),
                        start=True,
                        stop=True,
                    )
                    if c == 1:
                        mm.ins.ldweights = False
                ex = exps.tile([P, QH], BF16, tag="ex", name=f"ex{i}")
                nc.scalar.activation(out=ex, in_=sc, func=AFT.Exp, scale=SCALE)
                ex_tiles[i] = ex

            def attn_v(i):
                qh, h, kb = items[i]
                j, po = h // 2, (h % 2) * DH
                q0 = qh * QH
                if kb == 0:
                    ao_tiles[(qh, h)] = aop.tile(
                        [DH + 1, QH], F32, tag="ao", name=f"ao{qh}_{h}"
                    )
                ao_ps = ao_tiles[(qh, h)]
                ex = ex_tiles.pop(i)
                for c in range(2):
                    mm = nc.tensor.matmul(
                        ao_ps[:, c * 512 : (c + 1) * 512],
                        lhsT=(v_sb[:, kb, h, :]),
                        rhs=(ex[:, c * 512 : (c + 1) * 512]),
                        start=(kb == 0),
                        stop=(kb == NT - 1),
                    )
                    if c == 1:
                        mm.ins.ldweights = False
                if kb == NT - 1:
                    # evict unnormalized accumulator so the PSUM tile frees early
                    ao_sb = nrm.tile([DH + 1, QH], F32, tag="ao_sb", name=f"aosb{i}")
                    nc.vector.tensor_copy(ao_sb, ao_ps)
                    # the very last unit normalizes in 128-token chunks so the
                    # final output-projection tiles can pipeline with it
                    nchunk = 8 if i == len(items) - 1 else 1
                    cw = QH // nchunk
                    for ch in range(nchunk):
                        cs = ch * cw
                        recip = nrm.tile(
                            [1, QH], F32, tag="recip", name=f"rc{i}_{ch}", bufs=2
                        )
                        nc.vector.reciprocal(
                            out=recip[:, 0:cw],
                            in_=ao_sb[DH : DH + 1, cs : cs + cw],
                        )
                        scr = dram.tile([1, QH], F32, tag="scr", name=f"scr{i}{ch}")
                        nc.sync.dma_start(out=scr[:, 0:cw], in_=recip[:, 0:cw])
                        rb = nrm.tile(
                            [DH, QH], F32, tag="rb", name=f"rb{i}_{ch}", bufs=2
                        )
                        nc.sync.dma_start(
                            out=rb[:, 0:cw],
                            in_=scr[0:1, 0:cw].to_broadcast((DH, cw)),
                        )
                        nc.vector.tensor_tensor(
                            out=aoT[j][po : po + DH, q0 + cs : q0 + cs + cw],
                            in0=ao_sb[0:DH, cs : cs + cw],
                            in1=rb[:, 0:cw],
                            op=ALU.mult,
                        )
                        if nchunk > 1:
                            outproj_tile(NT // 2 + ch)

            def outproj_tile(mt):
                ps = opp.tile([P, D], F32, tag="o", name=f"o{mt}")
                for kt in range(2):
                    nc.tensor.matmul(
                        ps,
                        lhsT=(aoT[kt][:, mt * P : (mt + 1) * P]),
                        rhs=(w_o_sb[:, kt, :]),
                        start=(kt == 0),
                        stop=(kt == 1),
                    )
                ot = osb.tile([P, D], F32, tag="ot", name=f"ot{mt}")
                nc.vector.tensor_copy(ot, ps)
                nc.sync.dma_start(out=out[mt * P : (mt + 1) * P, :], in_=ot)

            DEPTH = 2
            for i in range(min(DEPTH, len(items))):
                sc_exp(i)
            for i in range(len(items)):
                if i + DEPTH < len(items):
                    sc_exp(i + DEPTH)
                attn_v(i)
                if items[i] == (0, HPC - 1, NT - 1):
                    for mt in range(NT // 2):
                        outproj_tile(mt)

    nc.compile()
    return nc


_NC_CACHE = None
_LAST_RESULT = None


def kernel(x, ln_scale, ln_bias, w_qkv, w_out):
    global _NC_CACHE, _LAST_RESULT
    if _NC_CACHE is None:
        _NC_CACHE = build_kernel()
    nc = _NC_CACHE

    import ml_dtypes

    x = np.asarray(x, np.float32)
    w_eff = (np.asarray(ln_scale, np.float32)[:, None] * np.asarray(w_qkv, np.float32))
    b_row = np.asarray(ln_bias, np.float32) @ np.asarray(w_qkv, np.float32)
    w_eff = w_eff.astype(ml_dtypes.bfloat16)
    w_out = np.asarray(w_out, np.float32).astype(ml_dtypes.bfloat16)

    in_maps = []
    for c in range(8):
        b, g = c // 2, c % 2
        s = slice(FPC * g, FPC * g + FPC)
        ks = slice(512 + FPC * g, 512 + FPC * g + FPC)
        vs = slice(1024 + FPC * g, 1024 + FPC * g + FPC)
        in_maps.append(
            {
                "xb": np.ascontiguousarray(x[b]),
                "wq": np.ascontiguousarray(w_eff[:, s]),
                "wk": np.ascontiguousarray(w_eff[:, ks]),
                "wv": np.ascontiguousarray(w_eff[:, vs]),
                "wo": np.ascontiguousarray(w_out[s, :]),
                "bq": np.ascontiguousarray(b_row[s]),
                "bk": np.ascontiguousarray(b_row[ks]),
                "bv": np.ascontiguousarray(b_row[vs]),
            }
        )
    res = run_bass_kernel_spmd(nc, in_maps, core_ids=list(range(8)))
    _LAST_RESULT = res
    outs = [res.results[c]["out"] for c in range(8)]
    return np.stack([outs[2 * b] + outs[2 * b + 1] for b in range(B)]).astype(
        np.float32
    )


if __name__ == "__main__":
    xs = np.random.randn(B, N, D).astype(np.float32)
    o = kernel(
        x=xs,
        ln_scale=np.ones(D, np.float32),
        ln_bias=np.zeros(D, np.float32),
        w_qkv=(np.random.randn(D, 3 * H * DH) / np.sqrt(D)).astype(np.float32),
        w_out=(np.random.randn(H * DH, D) / np.sqrt(H * DH)).astype(np.float32),
    )
    print(o.shape, o.dtype)


# revision 56
# speedup vs baseline: 1.0150x; 1.0150x over previous
"""Trainium2 Bass kernel for pre-LN multi-head self-attention.

Module: y = LN(x); qkv = y @ w_qkv; attention(8 heads, dh=64); out = ao @ w_out
Shapes: x [4, 2048, 512], w_qkv [512, 1536], w_out [512, 512], fp32.

Sharding (8 cores): core c -> batch b = c//2, head-group g = c%2 (4 heads).
Each core computes LN + QKV (its head slice) + attention + a partial output
projection (its heads' rows of w_out); the host sums the two partials per batch.

Per-core dataflow (transpose-free except one PE transpose of y):
  LN in natural [tok, d] layout (bn_stats) -> PE-transpose y -> yT [d, tok]
  Q^T, K^T = w^T @ yT   (features on partitions -- natural lhsT layout)
  V natural [tok, feat] with a fused ones-column so attn@V also accumulates
  the softmax denominator (row 64 of the PSUM accumulator).
  scoresT [k, q] = K^T.T @ Q^T per 128-k-token block; exp on ACT (no max
  subtraction needed: fp32, well-scaled inputs); attn@V accumulates over
  k-blocks in PSUM; per-head 1/sumexp broadcast via a DRAM roundtrip DMA;
  output projection consumes aoT directly as lhsT.
ln_scale/ln_bias are folded into w_qkv on the host (w_eff = scale*W,
bias_row = bias@W added per-feature on device), so the device LN is pure
normalize.  Matmul operands are bf16 (PSUM accumulation stays fp32); the
second matmul of each same-weight pair sets ldweights=False to skip the
redundant PE weight reload.  Stage D runs a depth-2 software pipeline
(scores+exp for item i+2 issue before attn@V of item i) so the in-order
PE never stalls on the ACT exp; the output projection is interleaved
per q-half, and the final unit normalizes in 128-token chunks so the
last projection tiles pipeline with it.
"""

import sys

if "/opt/trn_rl_repo" not in sys.path:
    sys.path.insert(0, "/opt/trn_rl_repo")

from contextlib import ExitStack

import numpy as np

import concourse.bass as bass
import concourse.tile as tile
from concourse.masks import make_identity
from concourse import bacc, mybir
from concourse.bass_utils import run_bass_kernel_spmd

B, N, D = 4, 2048, 512
H, DH = 8, 64
HPC = 4                 # heads per core
FPC = HPC * DH          # 256 features per core
P = 128
NT = N // P             # 16 token tiles
DT = D // P             # 4 d tiles
NQ = N // 512           # 4 q-blocks of 512
EPS = 1e-6
SCALE = DH ** -0.5
F32 = mybir.dt.float32
F32R = mybir.dt.float32r
BF16 = mybir.dt.bfloat16
ALU = mybir.AluOpType
AFT = mybir.ActivationFunctionType




def build_kernel():
    nc = bacc.Bacc("TRN2", target_bir_lowering=False, debug=False)
    xb = nc.dram_tensor("xb", [N, D], F32, kind="ExternalInput").ap()
    wq = nc.dram_tensor("wq", [D, FPC], BF16, kind="ExternalInput").ap()
    wk = nc.dram_tensor("wk", [D, FPC], BF16, kind="ExternalInput").ap()
    wv = nc.dram_tensor("wv", [D, FPC], BF16, kind="ExternalInput").ap()
    wo = nc.dram_tensor("wo", [FPC, D], BF16, kind="ExternalInput").ap()
    bq = nc.dram_tensor("bq", [FPC], F32, kind="ExternalInput").ap()
    bk = nc.dram_tensor("bk", [FPC], F32, kind="ExternalInput").ap()
    bv = nc.dram_tensor("bv", [FPC], F32, kind="ExternalInput").ap()
    out = nc.dram_tensor("out", [N, D], F32, kind="ExternalOutput").ap()

    with tile.TileContext(nc, pool_alloc_mode="queue") as tc, ExitStack() as ctx:
        consts = ctx.enter_context(tc.tile_pool(name="consts", bufs=1))
        big = ctx.enter_context(tc.tile_pool(name="big", bufs=1))
        dram = ctx.enter_context(tc.tile_pool(name="dram", bufs=2, space="DRAM"))

        identity = consts.tile([P, P], BF16)
        make_identity(nc, identity)
        eps_t = consts.tile([P, 1], F32)
        nc.vector.memset(eps_t, EPS)

        yT = [big.tile([P, N], BF16, tag=f"yT{j}", name=f"yT{j}") for j in range(DT)]
        qT = [big.tile([P, N], BF16, tag=f"qT{j}", name=f"qT{j}") for j in range(2)]
        kT = [big.tile([P, N], BF16, tag=f"kT{j}", name=f"kT{j}") for j in range(2)]
        aoT = [big.tile([P, N], BF16, tag=f"aoT{j}", name=f"aoT{j}") for j in range(2)]
        v_sb = big.tile([P, NT, HPC, DH + 1], BF16)
        ones_col = consts.tile([P, 1], F32)
        nc.vector.memset(ones_col, 1.0)
        nc.vector.tensor_copy(
            v_sb[:, :, :, DH : DH + 1],
            ones_col[:, 0:1].to_broadcast((P, NT, HPC, 1)),
        )

        # ---- Stage A+B: LayerNorm + transpose y -> yT ----
        with tc.tile_pool(name="ln", bufs=3) as ln, tc.tile_pool(
            name="tp_psum", bufs=4, space="PSUM"
        ) as tpp:
            for ig in range(NT // 4):  # groups of 4 token tiles
                y_ts = []
                for ii in range(4):
                    i = ig * 4 + ii
                    x_t = ln.tile([P, D], F32, tag="x")
                    nc.sync.dma_start(out=x_t, in_=xb[i * P : (i + 1) * P, :])
                    stats = ln.tile([P, 6], F32, tag="stats")
                    nc.vector.bn_stats(out=stats, in_=x_t)
                    mv = ln.tile([P, 2], F32, tag="mv")
                    nc.vector.bn_aggr(out=mv, in_=stats)
                    std = ln.tile([P, 1], F32, tag="std")
                    nc.scalar.activation(
                        out=std, in_=mv[:, 1:2], func=AFT.Sqrt, bias=eps_t[:, 0:1]
                    )
                    rstd = ln.tile([P, 1], F32, tag="rstd")
                    nc.vector.reciprocal(out=rstd, in_=std)
                    y_t = ln.tile([P, D], BF16, tag="y", bufs=6)
                    nc.vector.tensor_scalar(
                        out=y_t,
                        in0=x_t,
                        scalar1=mv[:, 0:1],
                        scalar2=rstd[:, 0:1],
                        op0=ALU.subtract,
                        op1=ALU.mult,
                    )
                    y_ts.append(y_t)
                for j in range(DT):
                    pt = tpp.tile([P, 512], BF16, tag="tp")
                    for ii in range(4):
                        nc.tensor.transpose(
                            pt[:, ii * P : (ii + 1) * P],
                            y_ts[ii][:, j * P : (j + 1) * P],
                            identity,
                        )
                    nc.scalar.activation(
                        out=yT[j][:, ig * 512 : (ig + 1) * 512],
                        in_=pt,
                        func=AFT.Copy,
                    )

        # weights: [d, f] -> sbuf [p, dt, f]
        w_q_sb = consts.tile([P, DT, FPC], BF16)
        nc.sync.dma_start(out=w_q_sb, in_=wq.rearrange("(t p) f -> p t f", p=P))
        w_k_sb = consts.tile([P, DT, FPC], BF16)
        nc.sync.dma_start(out=w_k_sb, in_=wk.rearrange("(t p) f -> p t f", p=P))
        w_v_sb = consts.tile([P, DT, FPC], BF16)
        nc.sync.dma_start(out=w_v_sb, in_=wv.rearrange("(t p) f -> p t f", p=P))
        w_o_sb = consts.tile([P, 2, D], BF16)
        nc.sync.dma_start(out=w_o_sb, in_=wo.rearrange("(t p) f -> p t f", p=P))
        bq_sb = consts.tile([P, 2], F32)
        nc.sync.dma_start(out=bq_sb, in_=bq.rearrange("(t p) -> p t", p=P))
        bk_sb = consts.tile([P, 2], F32)
        nc.sync.dma_start(out=bk_sb, in_=bk.rearrange("(t p) -> p t", p=P))
        bv_b = consts.tile([P, FPC], F32)
        bv_bcast = bass.AP(tensor=bv.tensor, offset=bv.offset, ap=[[0, P]] + list(bv.ap))
        nc.sync.dma_start(out=bv_b, in_=bv_bcast)

        # ---- Stage C: QKV projections ----
        with tc.tile_pool(name="c_psum", bufs=1, space="PSUM") as cpp, tc.tile_pool(
            name="v_psum", bufs=2, space="PSUM"
        ) as vpp:
            # j=0 groups first so heads 0/1 attention can start early; each
            # weight tile is loaded once and reused across the 4 token groups
            def qk_group(w_sb, b_sb, dstT, j):
                pss = [
                    cpp.tile([P, 512], F32, tag=f"qk{nt}", name=f"qk{nt}_{j}")
                    for nt in range(NQ)
                ]
                for dt in range(DT):
                    for nt in range(NQ):
                        mm = nc.tensor.matmul(
                            pss[nt],
                            lhsT=(w_sb[:, dt, j * P : (j + 1) * P]),
                            rhs=(yT[dt][:, nt * 512 : (nt + 1) * 512]),
                            start=(dt == 0),
                            stop=(dt == DT - 1),
                        )
                        if nt > 0:
                            mm.ins.ldweights = False
                for nt in range(NQ):
                    nc.scalar.activation(
                        out=dstT[j][:, nt * 512 : (nt + 1) * 512],
                        in_=pss[nt],
                        func=AFT.Identity,
                        bias=b_sb[:, j : j + 1],
                    )

            def v_group(irange):
                for i in irange:
                    ps = vpp.tile([P, FPC], F32, tag="v", name=f"v{i}")
                    for dt in range(DT):
                        nc.tensor.matmul(
                            ps,
                            lhsT=(yT[dt][:, i * P : (i + 1) * P]),
                            rhs=(w_v_sb[:, dt, :]),
                            start=(dt == 0),
                            stop=(dt == DT - 1),
                        )
                    nc.vector.tensor_tensor(
                        out=v_sb[:, i, :, 0:DH],
                        in0=ps.rearrange("p (h d) -> p h d", h=HPC),
                        in1=bv_b.rearrange("p (h d) -> p h d", h=HPC),
                        op=ALU.add,
                    )

            qk_group(w_k_sb, bk_sb, kT, 0)
            qk_group(w_q_sb, bq_sb, qT, 0)
            v_group(range(0, 8))
            qk_group(w_k_sb, bk_sb, kT, 1)
            qk_group(w_q_sb, bq_sb, qT, 1)
            v_group(range(8, NT))

        # ---- Stage D: attention, units of (q-half, head) pipelined ----
        QH = 1024
        with tc.tile_pool(name="sc_psum", bufs=2, space="PSUM") as scp, tc.tile_pool(
            name="ao_psum", bufs=1, space="PSUM"
        ) as aop, tc.tile_pool(name="exp_sb", bufs=6) as exps, tc.tile_pool(
            name="nrm", bufs=3
        ) as nrm, tc.tile_pool(
            name="o_psum", bufs=2, space="PSUM"
        ) as opp, tc.tile_pool(name="o_sb", bufs=3) as osb:
            items = [
                (qh, h, kb) for qh in range(2) for h in range(HPC) for kb in range(NT)
            ]
            ex_tiles = {}
            ao_tiles = {}

            def sc_exp(i):
                qh, h, kb = items[i]
                j, po = h // 2, (h % 2) * DH
                q0 = qh * QH
                sc = scp.tile([P, QH], F32, tag="sc", name=f"sc{i}")
                for c in range(2):
                    mm = nc.tensor.matmul(
                        sc[:, c * 512 : (c + 1) * 512],
                        lhsT=(kT[j][po : po + DH, kb * P : (kb + 1) * P]),
                        rhs=(qT[j][po : po + DH, q0 + c * 512 : q0 + (c + 1) * 512]),
                        start=True,
                        stop=True,
                    )
                    if c == 1:
                        mm.ins.ldweights = False
                ex = exps.tile([P, QH], BF16, tag="ex", name=f"ex{i}")
                nc.scalar.activation(out=ex, in_=sc, func=AFT.Exp, scale=SCALE)
                ex_tiles[i] = ex

            def attn_v(i):
                qh, h, kb = items[i]
                j, po = h // 2, (h % 2) * DH
                q0 = qh * QH
                if kb == 0:
                    ao_tiles[(qh, h)] = aop.tile(
                        [DH + 1, QH], F32, tag="ao", name=f"ao{qh}_{h}"
                    )
                ao_ps = ao_tiles[(qh, h)]
                ex = ex_tiles.pop(i)
                for c in range(2):
                    mm = nc.tensor.matmul(
                        ao_ps[:, c * 512 : (c + 1) * 512],
                        lhsT=(v_sb[:, kb, h, :]),
                        rhs=(ex[:, c * 512 : (c + 1) * 512]),
                        start=(kb == 0),
                        stop=(kb == NT - 1),
                    )
                    if c == 1:
                        mm.ins.ldweights = False
                if kb == NT - 1:
                    # evict unnormalized accumulator so the PSUM tile frees early
                    ao_sb = nrm.tile([DH + 1, QH], F32, tag="ao_sb", name=f"aosb{i}")
                    nc.vector.tensor_copy(ao_sb, ao_ps)
                    # the very last unit normalizes in 128-token chunks so the
                    # final output-projection tiles can pipeline with it
                    nchunk = 8 if i == len(items) - 1 else 1
                    cw = QH // nchunk
                    for ch in range(nchunk):
                        cs = ch * cw
                        recip = nrm.tile(
                            [1, QH], F32, tag="recip", name=f"rc{i}_{ch}", bufs=2
                        )
                        nc.vector.reciprocal(
                            out=recip[:, 0:cw],
                            in_=ao_sb[DH : DH + 1, cs : cs + cw],
                        )
                        scr = dram.tile([1, QH], F32, tag="scr", name=f"scr{i}{ch}")
                        nc.sync.dma_start(out=scr[:, 0:cw], in_=recip[:, 0:cw])
                        rb = nrm.tile(
                            [DH, QH], F32, tag="rb", name=f"rb{i}_{ch}", bufs=2
                        )
                        nc.sync.dma_start(
                            out=rb[:, 0:cw],
                            in_=scr[0:1, 0:cw].to_broadcast((DH, cw)),
                        )
                        nc.vector.tensor_tensor(
                            out=aoT[j][po : po + DH, q0 + cs : q0 + cs + cw],
                            in0=ao_sb[0:DH, cs : cs + cw],
                            in1=rb[:, 0:cw],
                            op=ALU.mult,
                        )
                        if nchunk > 1:
                            outproj_tile(NT // 2 + ch)

            def outproj_tile(mt):
                ps = opp.tile([P, D], F32, tag="o", name=f"o{mt}")
                for kt in range(2):
                    nc.tensor.matmul(
                        ps,
                        lhsT=(aoT[kt][:, mt * P : (mt + 1) * P]),
                        rhs=(w_o_sb[:, kt, :]),
                        start=(kt == 0),
                        stop=(kt == 1),
                    )
                ot = osb.tile([P, D], F32, tag="ot", name=f"ot{mt}")
                nc.vector.tensor_copy(ot, ps)
                nc.sync.dma_start(out=out[mt * P : (mt + 1) * P, :], in_=ot)

            DEPTH = 2
            for i in range(min(DEPTH, len(items))):
                sc_exp(i)
            for i in range(len(items)):
                if i + DEPTH < len(items):
                    sc_exp(i + DEPTH)
                attn_v(i)
                if items[i] == (0, HPC - 1, NT - 1):
                    for mt in range(NT // 2):
                        outproj_tile(mt)

    nc.compile()
    return nc


_NC_CACHE = None
_LAST_RESULT = None


def kernel(x, ln_scale, ln_bias, w_qkv, w_out):
    global _NC_CACHE, _LAST_RESULT
    if _NC_CACHE is None:
        _NC_CACHE = build_kernel()
    nc = _NC_CACHE

    import ml_dtypes

    x = np.asarray(x, np.float32)
    w_eff = (np.asarray(ln_scale, np.float32)[:, None] * np.asarray(w_qkv, np.float32))
    b_row = np.asarray(ln_bias, np.float32) @ np.asarray(w_qkv, np.float32)
    w_eff = w_eff.astype(ml_dtypes.bfloat16)
    w_out = np.asarray(w_out, np.float32).astype(ml_dtypes.bfloat16)

    in_maps = []
    for c in range(8):
        b, g = c // 2, c % 2
        s = slice(FPC * g, FPC * g + FPC)
        ks = slice(512 + FPC * g, 512 + FPC * g + FPC)
        vs = slice(1024 + FPC * g, 1024 + FPC * g + FPC)
        in_maps.append(
            {
                "xb": np.ascontiguousarray(x[b]),
                "wq": np.ascontiguousarray(w_eff[:, s]),
                "wk": np.ascontiguousarray(w_eff[:, ks]),
                "wv": np.ascontiguousarray(w_eff[:, vs]),
                "wo": np.ascontiguousarray(w_out[s, :]),
                "bq": np.ascontiguousarray(b_row[s]),
                "bk": np.ascontiguousarray(b_row[ks]),
                "bv": np.ascontiguousarray(b_row[vs]),
            }
        )
    res = run_bass_kernel_spmd(nc, in_maps, core_ids=list(range(8)))
    _LAST_RESULT = res
    outs = [res.results[c]["out"] for c in range(8)]
    return np.stack([outs[2 * b] + outs[2 * b + 1] for b in range(B)]).astype(
        np.float32
    )


if __name__ == "__main__":
    xs = np.random.randn(B, N, D).astype(np.float32)
    o = kernel(
        x=xs,
        ln_scale=np.ones(D, np.float32),
        ln_bias=np.zeros(D, np.float32),
        w_qkv=(np.random.randn(D, 3 * H * DH) / np.sqrt(D)).astype(np.float32),
        w_out=(np.random.randn(H * DH, D) / np.sqrt(H * DH)).astype(np.float32),
    )
    print(o.shape, o.dtype)
